# revision 3
# baseline (speedup 1.0000x reference)
"""Bilinear pooling kernel for 8 Trainium2 NeuronCores (Bass/Tile).

Computes out[b,n,v,o] = sum_{d,e} node[b,n,d] * veh[b,v,e] * W[o, d*E+e] + bias[o]
for B=16, N=64, V=16, D=E=128, O=256.  Tensor-sharded over O (32 ch/core).

  Stage A:  U[d, (o,b,v)] = sum_e W3[o,d,e] * veh[b,v,e]
  Stage B:  out[b][n, (o,v)] = sum_d node[b,n,d] * U[d, o-half, b, v]

Schedule model (learned from traces):
  - exec time = (tile teardown time) + fixed ~7.7us NRT semaphore-reset
    epilogue; so everything reduces to finishing kernel work ASAP.
  - PSUM evacuation (vector+scalar only; gpsimd can't touch PSUM) is the
    binding resource: 24 copies x ~640ns / 2 engines ~= 7.7us per engine.
  - Stage A is copy-paced at ~2ch/650ns ~= 100GB/s of weight consumption;
    inputs are chunked so each chunk's completion (bytes + ~0.8us receipt)
    beats its consumption time: front chunks ride the sync ring in
    consumption order; the scalar/gpsimd rings are paced by real WAW deps
    (1-elem writes chained off the act-table load) so they don't steal
    HBM bandwidth from the first chunks.
  - PE clock gate: 1.2 -> 2.4 GHz after ~4.7us of continuous PE activity,
    reset by multi-us idle gaps.  Pre-tile + in-tile warmup matmuls keep
    the PE busy from ~6.4us until the first stage-A matmul.
  - Output DMA completion receipts (~2us) must not gate the teardown
    barrier: the last three output DMAs issue after the TileContext
    closes, on three different engines, and drain under the NRT epilogue.
"""

import sys

import numpy as np

sys.path.insert(0, "/opt/trn_rl_repo")

B, N, V = 16, 64, 16
D = 128
E = 128
O = 256
NCORES = 8
OS = O // NCORES  # 32 output channels per core

PRE_WARM = 4  # 256-col warmups before the tile-entry barrier
WARM = 8  # in-tile warmups bridging to the first input chunk (~9.2us)

_nc_cache = {}


def _build():
    from contextlib import ExitStack

    import concourse.tile as tile
    from concourse import bacc, mybir

    f32 = mybir.dt.float32
    bf16 = mybir.dt.bfloat16

    nc = bacc.Bacc("TRN2", target_bir_lowering=False)
    # Input chunks in stage-A consumption order ch0..ch31.  Coarse front
    # chunks: each chunk's completion sem fires ~0.8-2us after its last
    # byte, so early channels must share few sems.  Ring FIFO serializes
    # transfers within a ring; pacing WAW deps keep the scalar/gpsimd
    # rings off the HBM until the sync ring's front chunks are in flight.
    #   sync ring:   c0 = [vehT | ch0-5], c1 = ch6-13, c4 = ch14-21 (FIFO)
    #   scalar ring: c5 = ch22-27                 (paced)
    #   gpsimd ring: nodeT, c6 = ch28-31          (paced)
    c0_d = nc.dram_tensor("c0", [E, B * V + 6 * D], bf16, kind="ExternalInput")
    c1_d = nc.dram_tensor("c1", [E, 8 * D], bf16, kind="ExternalInput")
    c4_d = nc.dram_tensor("c4", [E, 8 * D], bf16, kind="ExternalInput")
    c5_d = nc.dram_tensor("c5", [E, 6 * D], bf16, kind="ExternalInput")
    c6_d = nc.dram_tensor("c6", [E, 4 * D], bf16, kind="ExternalInput")
    nodeT_d = nc.dram_tensor("nodeT", [D, B * N], bf16, kind="ExternalInput")
    out_d = nc.dram_tensor("out", [4, 128, 1024], bf16, kind="ExternalOutput")

    # Raw tensors: warmup operands + post-tile output staging.
    warm_sb = nc.alloc_sbuf_tensor("warm_sb", [E, B * V], bf16)
    wps = nc.alloc_psum_tensor("wps", [D, 2, B * V], f32)
    ob1s = nc.alloc_sbuf_tensor("ob1s", [128, 2, 512], bf16)
    ob2s = nc.alloc_sbuf_tensor("ob2s", [128, 2, 512], bf16)
    ob3s = nc.alloc_sbuf_tensor("ob3s", [128, 2, 512], bf16)

    # Pre-tile warmup: PE activity from ~6.4us (right after the framework
    # preamble), so the clock gate ramps as early as possible.
    for i in range(PRE_WARM):
        nc.tensor.matmul(
            wps.ap()[:, i % 2], warm_sb.ap()[:, 0:D], warm_sb.ap(),
            start=True, stop=True,
        )

    with ExitStack() as ctx:
        tc = ctx.enter_context(tile.TileContext(nc))
        const = ctx.enter_context(tc.tile_pool(name="const", bufs=1))
        upool = ctx.enter_context(tc.tile_pool(name="u", bufs=2))
        psA = ctx.enter_context(tc.tile_pool(name="psA", bufs=3, space="PSUM"))
        psB = ctx.enter_context(tc.tile_pool(name="psB", bufs=4, space="PSUM"))
        outp = ctx.enter_context(tc.tile_pool(name="outp", bufs=1))

        # ---- input DMAs ----
        c0 = const.tile([E, B * V + 6 * D], bf16)
        nc.sync.dma_start(c0[:], c0_d[:])
        c1 = const.tile([E, 8 * D], bf16)
        nc.sync.dma_start(c1[:], c1_d[:])
        # 1-element scalar ACTIVATE: waits on the 1.3us act-table load; the
        # paced triggers below chain off it via real WAW deps so the
        # scheduler cannot hoist them into the front chunks' bandwidth.
        c4 = const.tile([E, 8 * D], bf16)
        nc.sync.dma_start(c4[:], c4_d[:])
        actwarm = const.tile([E, 2], bf16)
        nc.scalar.copy(actwarm[:, 0:1], actwarm[:, 1:2])
        c5 = const.tile([E, 6 * D], bf16)
        nc.scalar.copy(c5[:, 0:1], actwarm[:, 0:1])  # pacing dep
        nc.scalar.dma_start(c5[:], c5_d[:])
        nodeT_t = const.tile([D, B * N], bf16)
        nc.gpsimd.tensor_copy(nodeT_t[:, 0:1], actwarm[:, 0:1])  # pacing dep
        nc.gpsimd.dma_start(nodeT_t[:], nodeT_d[:])
        c6 = const.tile([E, 4 * D], bf16)
        nc.gpsimd.tensor_copy(c6[:, 0:1], actwarm[:, 0:1])  # pacing dep
        nc.gpsimd.dma_start(c6[:], c6_d[:])
        nodeT = nodeT_t[:]
        vehT = c0[:, : B * V]

        # channel -> chunk mapping: c0 ch0-5 (after vehT), c1 ch6-13,
        # c4 ch14-21, c5 ch22-27, c6 ch28-31
        def wsel(o):
            if o < 6:
                return c0[:, B * V + o * D : B * V + (o + 1) * D]
            if o < 14:
                return c1[:, (o - 6) * D : (o - 5) * D]
            if o < 22:
                return c4[:, (o - 14) * D : (o - 13) * D]
            if o < 28:
                return c5[:, (o - 22) * D : (o - 21) * D]
            return c6[:, (o - 28) * D : (o - 27) * D]

        # In-tile warmups on the raw tensors: no deps, PE picks them up
        # right after the entry branch.
        for i in range(WARM):
            nc.tensor.matmul(
                wps.ap()[:, i % 2], warm_sb.ap()[:, 0:D], warm_sb.ap(),
                start=True, stop=True,
            )

        U = [
            upool.tile([D, 16, B, V], bf16, tag="U", name=f"U{h}") for h in range(2)
        ]

        vcp = nc.vector.tensor_copy
        scp = nc.scalar.copy

        def stageA(g):
            # 4 channels as two 2-channel/1-bank psum units; copies
            # alternate vector/scalar
            h, gl = divmod(g, 4)
            for u in range(2):
                pa = psA.tile([D, 2, B * V], f32, tag="pa", name=f"pa{g}{u}")
                for i in range(2):
                    nc.tensor.matmul(
                        pa[:, i], wsel(4 * g + 2 * u + i), vehT, start=True, stop=True
                    )
                ceng = vcp if u == 0 else scp
                c0_ = 4 * gl + 2 * u
                ceng(U[h][:, c0_ : c0_ + 2, :, :], pa[:])

        def stageB(h, q, ob, ceng, dma):
            # batches 4q..4q+3 of o-half h -> psum [128, 2, 256] -> ob slot
            pb = psB.tile([N * 2, 2, 256], f32, tag="pb")
            for j in range(2):
                for pbi in range(2):
                    b = 4 * q + 2 * j + pbi
                    nc.tensor.matmul(
                        pb[64 * pbi : 64 * (pbi + 1), j],
                        nodeT[:, b * N : (b + 1) * N],
                        U[h][:, :, b, :],
                        start=True,
                        stop=True,
                    )
            ceng(ob[:, q % 2], pb[:])
            # "full": 256KB tile DMA; "half": this q's 128KB only
            if dma == "full":
                nc.sync.dma_start(out_d[2 * h + q // 2], ob[:])
            elif dma == "half":
                s = (q % 2) * 512
                nc.sync.dma_start(
                    out_d[2 * h + q // 2][:, s : s + 512], ob[:, q % 2]
                )

        # pipeline: A g0..g4, then B(h0) with A g5-g7 interleaved, B(h1).
        # ob0 DMAs in-tile (receipts clear before teardown); ob1's second
        # half + ob2 + ob3 drain post-tile under the NRT epilogue.
        for g in range(5):
            stageA(g)
        ob0 = outp.tile([128, 2, 512], bf16, name="ob0")
        stageB(0, 0, ob0, vcp, None)
        stageA(5)
        stageB(0, 1, ob0, scp, "full")
        stageA(6)
        stageB(0, 2, ob1s.ap(), vcp, "half")
        stageA(7)
        stageB(0, 3, ob1s.ap(), scp, None)
        stageB(1, 0, ob2s.ap(), vcp, None)
        stageB(1, 1, ob2s.ap(), scp, None)
        stageB(1, 2, ob3s.ap(), vcp, None)
        stageB(1, 3, ob3s.ap(), scp, None)

    # Post-tile: ordered after all copies by the teardown barrier; the
    # transfers overlap the NRT epilogue.  Three different engines so the
    # triggers themselves run in parallel.
    psem1 = nc.alloc_semaphore("post_dma1")
    psem2 = nc.alloc_semaphore("post_dma2")
    psem3 = nc.alloc_semaphore("post_dma3")
    nc.gpsimd.dma_start(out_d[1][:, 512:1024], ob1s.ap()[:, 1]).then_inc(psem1, 16)
    nc.scalar.dma_start(out_d[2], ob2s.ap()).then_inc(psem2, 16)
    nc.sync.dma_start(out_d[3], ob3s.ap()).then_inc(psem3, 16)

    nc.compile()
    return nc


def _get_nc():
    if "nc" not in _nc_cache:
        _nc_cache["nc"] = _build()
    return _nc_cache["nc"]


def _prep_inputs(node_embed, veh_fea, W, b):
    import ml_dtypes

    def cast(x):
        return np.ascontiguousarray(x.astype(ml_dtypes.bfloat16))

    node_embed = np.asarray(node_embed, dtype=np.float32)
    veh_fea = np.asarray(veh_fea, dtype=np.float32)
    W = np.asarray(W, dtype=np.float32)

    nodeT = cast(node_embed.transpose(2, 0, 1).reshape(D, B * N))
    vehT = cast(veh_fea.transpose(2, 0, 1).reshape(E, B * V))
    W3 = W.reshape(O, D, E)

    in_maps = []
    for c in range(NCORES):
        # [E, o_local, D] channel-major weights for this core's O-shard
        wtc = W3[c * OS : (c + 1) * OS].transpose(2, 0, 1).reshape(E, OS * D)
        in_maps.append(
            {
                "c0": cast(np.concatenate([vehT, wtc[:, 0 : 6 * D]], axis=1)),
                "c1": cast(wtc[:, 6 * D : 14 * D]),
                "c4": cast(wtc[:, 14 * D : 22 * D]),
                "c5": cast(wtc[:, 22 * D : 28 * D]),
                "c6": cast(wtc[:, 28 * D : 32 * D]),
                "nodeT": nodeT,
            }
        )
    return in_maps


def run(node_embed, veh_fea, W, b, trace=False):
    from concourse.bass_utils import run_bass_kernel_spmd

    nc = _get_nc()
    in_maps = _prep_inputs(node_embed, veh_fea, W, b)
    res = run_bass_kernel_spmd(nc, in_maps, list(range(NCORES)), trace=trace)
    outs = []
    for r in res.results:
        # [4, 128, 1024] -> [h, qp, pb, n, j2, j, ch, v] -> [b, n, v, (h,ch)]
        # with b = 8*qp + 4*j2 + 2*j + pb
        arr = np.asarray(r["out"]).astype(np.float32)
        arr = arr.reshape(2, 2, 2, 64, 2, 2, 16, 16)
        arr = arr.transpose(1, 4, 5, 2, 3, 7, 0, 6).reshape(B, N, V, OS)
        outs.append(arr)
    full = np.concatenate(outs, axis=3) + np.asarray(b, np.float32)
    return np.ascontiguousarray(full, dtype=np.float32), res


def kernel(node_embed, veh_fea, W, b):
    return run(node_embed, veh_fea, W, b)[0]
